# revision 22
# baseline (speedup 1.0000x reference)
"""Trainium2 Bass kernel for nn_Attention_39934605918652.

res[b] = W0 @ x0[b] + sum_{n=1..N-1} W2 @ tanh(W1a @ x0[b] + W1b @ x[b,n])

Key algebraic optimization: W2 does not depend on n, so
    sum_n W2 @ tanh(...) = W2 @ (sum_n tanh(...))
which removes the second big matmul (only a [B,H]x[H,F] remains).

Sharding: data-parallel over batch B=128 across 8 cores (16 batches/core),
weights replicated. No collectives.

The dominant [F=512]-contraction matmul runs in fp8 e4m3 DoubleRow mode
(256 contraction rows per instruction; measured 215ns per 512-col matmul
warm = 2x bf16). W1b is host-scaled by 32 so its N(0, 1/1024) entries use
the e4m3 range; the tanh compensates via the ACT scale=1/32 immediate.

Engine budget (measured): the PE+ACT pair carries a conserved ~2.5us per
quad-tile (bias via ACT costs 4 small calls = 1.59us ACT; bias via a K=16
one-hot PE matmul costs 0.63us PE + 1.04us big-call ACT), so the mix knob
KB_NS1 balances them. ACT is not subject to the PE's HAM clock throttle,
so the default mix makes ACT the steady-state pacer. The segmented
free-dim reduce runs on DVE, with a GpSimd halving pre-add (SBUF-only;
GPSIMD cannot touch PSUM, cannot reduce, runs elementwise at 0.42 eff)
offloading KB_NGH of the 32 tiles.

All DRAM tensors are host-packed so every SBUF tile loads with ONE
contiguous dma_start of >=2KB-per-partition rows (small descriptors
measured ~40% DMA throughput loss):
  xiQ   [8*128, 2048] fp8   row (fp*4+q)*128+p = xi[f=fp*256+i*128+p,
                            q*1024+c] pairs (i,c)-major; pad col n=255
  w1bQ  [2*128, 2048] fp8   (= 32*W1b.T, DoubleRow pair layout per fp)
  x0T   [128, 4*16]   fp16  host-packed f-chunks side by side
  x0Q8  [128, 4*16]   fp8   same, for the fp8 W0-term matmuls
  w1aT  [512, 1024]   fp16  (= W1a.T)
  w2Q   [4*128, 1024] fp16  h-tile pairs side by side (= W2.T regrouped)
  w0Q   [128, 2048]   fp8   f-chunks side by side (= W0.T regrouped)
  bmask [16, 4*1024]  fp16  one-hot bias mask per quad: bmask[r, q*1024+
                            b*256+n] = 32 iff r == q*4+b and n != 255
Output res [BL=16, F=512] per core (batch-major); host concatenates.
"""

import os
import numpy as np
from contextlib import ExitStack

import concourse.bass as bass
import concourse.tile as tile
from concourse import bacc, mybir
from concourse.bass_utils import run_bass_kernel_spmd

N_CORES = 8
B, N, F, H = 128, 256, 512, 1024
BL = B // N_CORES          # 16 batches per core
NI = N - 1                 # 255 real columns per batch
NP = 256                   # padded columns per batch
NF = F // 128              # 4 f-chunks
FP = 2                     # 2 f-pair chunks (256 rows each, DoubleRow)
NH = H // 128              # 8 h-tiles
QUADS = BL // 4            # 4 batch-quads; per quad psum tile [128, 4*256]
QW = 4 * NP                # 1024 columns per quad
WSCALE = 32.0              # host-side W1b/bias scale (ACT scale=1/32)

F32 = mybir.dt.float32
BF16 = mybir.dt.bfloat16
F16 = mybir.dt.float16
F8 = mybir.dt.float8e4
DR = mybir.MatmulPerfMode.DoubleRow

# Knobs (sweepable on hw):
#  KB_NS1A: tiles using S1a = 4 per-batch fused-bias ACT calls with the
#          ACT accumulator producing S directly (no reduce; +187ns engine
#          time per call for the accumulator read).
#  KB_NS1: tiles using S1 = 4 per-batch fused-bias ACT calls + reduce.
#          Remaining tiles use S4 = PE one-hot bias matmul + one big
#          1024-col plain tanh + reduce. The last TAIL_S4 tiles are
#          forced S4 (single tanh call drains the pipeline fastest).
#  KB_NGH: reduces prefaced by a GpSimd halving add, then a half-width
#          DVE reduce. Rest: plain DVE reduce_sum.
#  KB_NTTR: DVE tensor_tensor_reduce variant — measured on hw: the
#          instruction wedges the device (NRT unrecoverable). Leave 0.
#  KB_WARM: dummy [128,128] f32 matmuls (426ns each: fp32 = 2 passes)
#          during the DMA lead-in to keep the PE clock governor warm.
#  KB_PPB: main PSUM pool bufs ([128,1024] f32 slots, 2 banks each).
NS1A = int(os.environ.get("KB_NS1A", "0"))
NS1 = int(os.environ.get("KB_NS1", "20"))
NS2D = int(os.environ.get("KB_NS2D", "0"))
NTTR = int(os.environ.get("KB_NTTR", "0"))
NGH = int(os.environ.get("KB_NGH", "24"))
WARM_N = int(os.environ.get("KB_WARM", "6"))
PPB = int(os.environ.get("KB_PPB", "4"))
TAIL_S4 = int(os.environ.get("KB_TAIL", "5"))


def _build_kernel():
    nc = bacc.Bacc(
        "TRN2", target_bir_lowering=False, debug=False, num_devices=N_CORES
    )

    xiQ = nc.dram_tensor("xiQ", [FP * QUADS * 128, 2048], F8, kind="ExternalInput").ap()
    w1bQ = nc.dram_tensor("w1bQ", [FP * 128, 2048], F8, kind="ExternalInput").ap()
    x0T = nc.dram_tensor("x0T", [128, NF * BL], F16, kind="ExternalInput").ap()
    x0Q8 = nc.dram_tensor("x0Q8", [128, NF * BL], F8, kind="ExternalInput").ap()
    w1aT = nc.dram_tensor("w1aT", [F, H], F16, kind="ExternalInput").ap()
    w2Q = nc.dram_tensor("w2Q", [NF * 128, 1024], F16, kind="ExternalInput").ap()
    w0Q = nc.dram_tensor("w0Q", [128, 2048], F8, kind="ExternalInput").ap()
    bmaskT = nc.dram_tensor(
        "bmaskT", [BL, QUADS * 1024], F16, kind="ExternalInput"
    ).ap()
    res = nc.dram_tensor("res", [BL, F], F32, kind="ExternalOutput").ap()

    with tile.TileContext(nc) as tc:
        with ExitStack() as ctx:
            _kernel_body(
                ctx, tc, xiQ, w1bQ, x0T, x0Q8, w1aT, w2Q, w0Q, bmaskT, res
            )

    nc.compile()
    return nc


def _kernel_body(ctx, tc, xiQ, w1bQ, x0T, x0Q8, w1aT, w2Q, w0Q, bmaskT, res):
    nc = tc.nc
    Tanh = mybir.ActivationFunctionType.Tanh

    wpool = ctx.enter_context(tc.tile_pool(name="weights", bufs=1))

    def load_rows(name, dram, r0, shape, dt):
        t = wpool.tile(shape, dt, tag=name, name=name)
        flat = t[:] if len(shape) == 2 else t[:].rearrange("p a b -> p (a b)")
        nc.sync.dma_start(flat, dram[r0 : r0 + shape[0], :])
        return t

    # ---- DMA issue order = first-need order: the very first tile's
    # operands stream before anything else so the PE starts ~9.5us.
    xi_sb = [[None] * QUADS for _ in range(FP)]
    w1b_sb = [None, None]
    w1b_sb[0] = load_rows("w1b_0", w1bQ, 0, [128, 2, 1024], F8)
    xi_sb[0][0] = load_rows("xi_0_0", xiQ, 0, [128, 2, 1024], F8)
    w1b_sb[1] = load_rows("w1b_1", w1bQ, 128, [128, 2, 1024], F8)
    xi_sb[1][0] = load_rows("xi_1_0", xiQ, QUADS * 128, [128, 2, 1024], F8)
    x0_all = load_rows("x0", x0T, 0, [128, NF * BL], F16)
    x0_sb = [x0_all[:, f * BL : (f + 1) * BL] for f in range(NF)]
    x08_all = load_rows("x08", x0Q8, 0, [128, NF * BL], F8)
    x08_sb = [x08_all[:, f * BL : (f + 1) * BL] for f in range(NF)]
    for fp in range(FP):
        xi_sb[fp][1] = load_rows(
            f"xi_{fp}_1", xiQ, (fp * QUADS + 1) * 128, [128, 2, 1024], F8
        )
    w1a_sb = [
        load_rows(f"w1a_{c}", w1aT, c * 128, [128, H], F16) for c in range(NF)
    ]
    bmask_sb = load_rows("bmask", bmaskT, 0, [BL, QUADS * 1024], F16)
    w0_sb = load_rows("w0", w0Q, 0, [128, 2048], F8)
    for q in (2, 3):  # wave-1 quads stream during wave 0
        for fp in range(FP):
            xi_sb[fp][q] = load_rows(
                f"xi_{fp}_{q}", xiQ, (fp * QUADS + q) * 128, [128, 2, 1024], F8
            )
    w2_sb = [
        load_rows(f"w2_{j}", w2Q, j * 128, [128, 1024], F16) for j in range(NF)
    ]

    def w2_slice(h):
        return w2_sb[h // 2][:, (h % 2) * 512 : (h % 2 + 1) * 512]

    h0_sb = [
        wpool.tile([128, BL], F32, tag=f"h0_{h}", name=f"h0_{h}")
        for h in range(NH)
    ]
    h0T_sb = wpool.tile([BL, H], F16, tag="h0T", name="h0T")
    h0s_sb = [
        wpool.tile([128, BL], F32, tag=f"h0s_{h}", name=f"h0s_{h}")
        for h in range(NH)
    ]
    S_sb = [
        wpool.tile([128, BL], F16, tag=f"S_{h}", name=f"S_{h}")
        for h in range(NH)
    ]

    # One PSUM pool; every tile shares the tag so slots recycle.
    # Slot = [128, 4*NP] f32 = 2 banks; PPB slots = the full 8 banks.
    # Warm-up, phase 1, and the epilogue matmul groups borrow slots
    # transiently; the epilogue accumulates in SBUF via DVE.
    ppool = ctx.enter_context(tc.tile_pool(name="ps", bufs=PPB, space="PSUM"))
    itpool = ctx.enter_context(tc.tile_pool(name="it", bufs=6))

    # ---- Phase 0: PE warm-up during the DMA lead-in ----
    if WARM_N:
        wz = wpool.tile([128, 128], F32, tag="warmz", name="warmz")
        nc.vector.memset(wz[:], 0.0)
        pw = ppool.tile([128, 128], F32, tag="ps", name="pwarm")
        for _ in range(WARM_N):
            nc.tensor.matmul(pw[:], wz[:], wz[:], start=True, stop=True)

    # ---- Phase 0b: preload the tanh ACT table during the DMA lead-in
    tiny = wpool.tile([128, 1], F32, tag="tiny", name="tiny")
    nc.vector.memset(tiny[:], 0.0)
    nc.scalar.activation(tiny[:], tiny[:], Tanh)

    # ---- Phase 1 (issued after the LEAD wave-0 tiles; see below):
    def phase1a():
        # h0[h*128+p, b] = sum_f W1a[h, f] x0[b, f]; [128, BL] per h-tile.
        for h in range(NH):
            ph = ppool.tile([128, BL], F32, tag="ps", name=f"ph0_{h}")
            for f in range(NF):
                nc.tensor.matmul(
                    ph[:],
                    w1a_sb[f][:, h * 128 : (h + 1) * 128],
                    x0_sb[f],
                    start=(f == 0),
                    stop=(f == NF - 1),
                )
            nc.vector.tensor_copy(h0_sb[h][:], ph[:])
            if NS2D:
                # 32*h0 for the S2d DVE bias path (ACT rescales by 1/32)
                nc.vector.tensor_scalar_mul(h0s_sb[h][:], ph[:], WSCALE)

    def phase1b():
        # h0T[b, h] flipped variant for the S4 bias matmuls' stationary.
        ph = ppool.tile([BL, H], F32, tag="ps", name="ph0T")
        for hb in range(2):
            for f in range(NF):
                nc.tensor.matmul(
                    ph[:, hb * 512 : (hb + 1) * 512],
                    x0_sb[f],
                    w1a_sb[f][:, hb * 512 : (hb + 1) * 512],
                    start=(f == 0),
                    stop=(f == NF - 1),
                )
        with nc.allow_low_precision(reason="h0T feeds fp16 bias matmul"):
            nc.vector.tensor_copy(h0T_sb[:], ph[:])

    # ---- Phase 3: epilogue res = W0 x0 + W2 S, accumulated in SBUF.
    # Each 4-matmul group borrows a psum slot transiently and DVE folds
    # it into rt_acc, so no slot is held across phase 2.
    rt_acc = wpool.tile([BL, F], F32, tag="rt", name="rt_acc")

    def epilogue_w0():
        pw = ppool.tile([BL, F], F32, tag="ps", name="po_w0")
        for f in range(NF):
            nc.tensor.matmul(
                pw[:],
                x08_sb[f],
                w0_sb[:, f * 512 : (f + 1) * 512],
                start=(f == 0),
                stop=(f == NF - 1),
            )
        nc.vector.tensor_copy(rt_acc[:], pw[:])

    def epilogue_s_group(hs, name):
        pg = ppool.tile([BL, F], F32, tag="ps", name=name)
        for i, h in enumerate(hs):
            nc.tensor.matmul(
                pg[:], S_sb[h][:], w2_slice(h),
                start=(i == 0), stop=(i == len(hs) - 1),
            )
        nc.vector.tensor_add(rt_acc[:], rt_acc[:], pg[:])

    # ---- Phase 2: hi matmul (fp8 DoubleRow) + bias + tanh + reduce ----
    def consume(h, q, pb, cls, red):
        it = itpool.tile([128, 4 * NP], BF16, tag="it", name=f"it_{h}_{q}")
        s1ish = cls in ("s1", "s1a")
        nb = NP if cls == "s4" else NI
        with nc.allow_low_precision(
            reason="S accumulated in 16-bit to feed the 16-bit output matmul"
        ):
            if s1ish:
                # per-batch ACT, bias via the ACT bias port, skip pad col;
                # s1a also reads the ACT accumulator = the row sum = S col.
                for bl in range(4):
                    b = q * 4 + bl
                    acc = S_sb[h][:, b : b + 1] if cls == "s1a" else None
                    nc.scalar.activation(
                        it[:, bl * NP : bl * NP + NI],
                        pb[:, bl * NP : bl * NP + NI],
                        Tanh,
                        bias=h0_sb[h][:, b : b + 1],
                        scale=1.0 / WSCALE,
                        accum_out=acc,
                    )
                if cls == "s1a":
                    return
            elif cls == "s2d":
                # S2d: bias on DVE (scalar_tensor_tensor, per-partition
                # scalar = 32*h0 column), then one big strided tanh call.
                for bl in range(4):
                    b = q * 4 + bl
                    # in1 is bypassed; walrus allows only one PSUM input,
                    # so point it at any resident SBUF tile.
                    nc.vector.scalar_tensor_tensor(
                        it[:, bl * NP : bl * NP + NI],
                        pb[:, bl * NP : bl * NP + NI],
                        h0s_sb[h][:, b : b + 1],
                        w1a_sb[0][:, :NI],
                        mybir.AluOpType.add,
                        mybir.AluOpType.bypass,
                    )
                sview = it[:].rearrange("p (b n) -> p b n", b=4)[:, :, :NI]
                nc.scalar.activation(sview, sview, Tanh, scale=1.0 / WSCALE)
            else:
                # S4: bias already in PSUM (one-hot matmul, pad col exact 0
                # since bmask zeroes it and tanh(0)=0): one big tanh call.
                nc.scalar.activation(it[:], pb[:], Tanh, scale=1.0 / WSCALE)
            scol = S_sb[h][:, q * 4 : (q + 1) * 4]
            view = it[:].rearrange("p (b n) -> p b n", b=4)
            if red == "gph":
                # GpSimd halving add (SBUF-only), then half-width DVE reduce.
                hb = nb // 2
                nc.gpsimd.tensor_add(
                    view[:, :, :hb],
                    view[:, :, :hb],
                    view[:, :, nb - hb : nb],
                )
                nc.vector.reduce_sum(
                    scol, view[:, :, : nb - hb], axis=mybir.AxisListType.X
                )
            else:
                nc.vector.reduce_sum(
                    scol, view[:, :, :nb], axis=mybir.AxisListType.X
                )

    def mm_main(pb, h, q, s1ish):
        # 2 DoubleRow matmuls per 512-col block: fpair 0 starts, fpair 1
        # accumulates; S4 groups stay open for the bias matmul.
        for bk in range(2):
            out = pb[:, bk * 512 : (bk + 1) * 512]
            for fp in range(FP):
                nc.tensor.matmul(
                    out,
                    w1b_sb[fp][:, :, h * 128 : (h + 1) * 128],
                    xi_sb[fp][q][:, :, bk * 512 : (bk + 1) * 512],
                    start=(fp == 0),
                    stop=(fp == FP - 1) and s1ish,
                    perf_mode=DR,
                )

    def mm_bias(pb, h, q):
        for bk in range(2):
            nc.tensor.matmul(
                pb[:, bk * 512 : (bk + 1) * 512],
                h0T_sb[:, h * 128 : (h + 1) * 128],
                bmask_sb[:, q * 1024 + bk * 512 : q * 1024 + (bk + 1) * 512],
                start=False,
                stop=True,
            )

    # PE program order: LEAD wave-0 tiles first (gated only on w1b +
    # xi-q0/q1, ~1MB of DMA), then phase 1 (w1a streams meanwhile), then
    # the rest. Wave 1 walks h DESCENDING so S[7..4] complete early and
    # their epilogue group issues mid-stream; only {3..0} trails the
    # final consume.
    # Wave 0 walks q-major so the first 8 tiles need only quad-0 xi data
    # (the q1 stream lands while they run); wave 1 h-descending as above.
    sched = []
    for q in (0, 1):
        for h in range(NH):
            sched.append((0, h, q))
    for h in range(NH - 1, -1, -1):
        for q in (2, 3):
            sched.append((1, h, q))

    LEAD = 3  # <= PPB-1: lead tiles hold slots; phase 1 borrows the last

    # Deficit-spread class labels; first LEAD tiles non-S4 (their matmuls
    # precede h0T in the PE stream), last TAIL_S4 tiles forced S4.
    counts = {"s1a": NS1A, "s1": NS1, "s2d": NS2D}
    nons4 = counts["s1a"] + counts["s1"] + counts["s2d"]
    if nons4 < LEAD:
        counts["s1"] += LEAD - nons4
    counts["s4"] = 32 - sum(counts.values())
    labels = []
    used = {k: 0 for k in counts}
    for pos in range(32):
        opts = [k for k in counts if used[k] < counts[k]]
        if pos < LEAD:
            opts = [k for k in opts if k != "s4"] or ["s1a"]
        elif pos >= 32 - TAIL_S4 and used["s4"] < counts["s4"]:
            opts = ["s4"]
        pick = max(opts, key=lambda k: counts[k] * (pos + 1) / 32 - used[k])
        used[pick] = used.get(pick, 0) + 1
        labels.append(pick)
    nred = sum(1 for l in labels if l != "s1a")
    rcounts = {"ttr": min(NTTR, nred), "gph": min(NGH, max(nred - NTTR, 0))}
    rcounts["plain"] = nred - rcounts["ttr"] - rcounts["gph"]
    rlabels = []
    rused = {k: 0 for k in rcounts}
    for pos in range(nred):
        opts = [k for k in rcounts if rused[k] < rcounts[k]]
        if pos >= nred - 4 and rused["plain"] < rcounts["plain"]:
            opts = ["plain"]
        pick = max(opts, key=lambda k: rcounts[k] * (pos + 1) / nred - rused[k])
        rused[pick] += 1
        rlabels.append(pick)
    riter = iter(rlabels)
    plan = [(l, next(riter) if l != "s1a" else None) for l in labels]

    deferred = []
    for pos, (wave, h, q) in enumerate(sched):
        if pos == LEAD:
            phase1a()
            phase1b()
            for dh, dq, dpb, dcls, dred in deferred:
                if dcls == "s4":
                    mm_bias(dpb, dh, dq)
                consume(dh, dq, dpb, dcls, dred)
        cls, red = plan[pos]
        pb = ppool.tile([128, 4 * NP], F32, tag="ps", name=f"pb_{h}_{q}")
        mm_main(pb, h, q, cls != "s4")
        if pos < LEAD:
            deferred.append((h, q, pb, cls, red))
            continue
        if cls == "s4":
            mm_bias(pb, h, q)
        consume(h, q, pb, cls, red)
        if pos == 11:
            epilogue_w0()
        if pos == 25:
            epilogue_s_group([7, 6, 5, 4], "po_sA")
        if pos == 29:
            epilogue_s_group([3, 2], "po_sB1")

    epilogue_s_group([1, 0], "po_sB2")
    nc.sync.dma_start(res[:], rt_acc[:])


_NC_CACHE = {}


def _get_nc():
    key = ("v9", NS1A, NS1, NS2D, NTTR, NGH, WARM_N, PPB, TAIL_S4)
    if key not in _NC_CACHE:
        _NC_CACHE[key] = _build_kernel()
    return _NC_CACHE[key]


def _make_in_maps(x, W1, W2, W0):
    import ml_dtypes

    f8 = ml_dtypes.float8_e4m3
    x = np.ascontiguousarray(np.asarray(x, dtype=np.float32))
    W1 = np.asarray(W1, dtype=np.float32)
    W2 = np.asarray(W2, dtype=np.float32)
    W0 = np.asarray(W0, dtype=np.float32)

    w1aT = np.ascontiguousarray(W1[:, :F].T).astype(np.float16)       # [F, H]
    w1bT = (W1[:, F:].T * WSCALE).astype(f8)                          # [F, H]
    # DoubleRow pair layout: row fp*128+p = [i0 h0..1023, i1 h0..1023]
    w1bQ = np.ascontiguousarray(
        w1bT.reshape(FP, 2, 128, H).transpose(0, 2, 1, 3).reshape(FP * 128, 2 * H)
    )
    w2T = np.ascontiguousarray(W2.T).astype(np.float16)               # [H, F]
    w2Q = np.ascontiguousarray(
        w2T.reshape(NF, 2, 128, F).transpose(0, 2, 1, 3).reshape(NF * 128, 2 * F)
    )
    w0T = np.ascontiguousarray(W0.T).astype(f8)                       # [F, F]
    w0Q = np.ascontiguousarray(
        w0T.reshape(NF, 128, F).transpose(1, 0, 2).reshape(128, NF * F)
    )

    # bmask[r, q*1024 + b*256 + n] = WSCALE iff r == q*4+b and n != 255
    bmask = np.zeros((BL, QUADS, 4, NP), dtype=np.float16)
    for qq in range(QUADS):
        for bb in range(4):
            bmask[qq * 4 + bb, qq, bb, :NI] = WSCALE
    bmask = bmask.reshape(BL, QUADS * 1024)

    in_maps = []
    for i in range(N_CORES):
        xc = x[i * BL : (i + 1) * BL]               # [BL, N, F]
        # packed [128, NF*BL]: row p, block f holds x0T[f*128+p, :]
        x0p = np.ascontiguousarray(
            xc[:, 0, :].T.reshape(NF, 128, BL).transpose(1, 0, 2).reshape(128, NF * BL)
        )
        pad = np.zeros((BL, NP, F), dtype=np.float32)
        pad[:, :NI, :] = xc[:, 1:, :]
        xiT = pad.reshape(BL * NP, F).T.astype(f8)  # [F, BL*NP]
        # row (fp*4+q)*128+p = [i0 c0..1023, i1 c0..1023] of quad q
        xiQ = np.ascontiguousarray(
            xiT.reshape(FP, 2, 128, QUADS, QW)
            .transpose(0, 3, 2, 1, 4)
            .reshape(FP * QUADS * 128, 2 * QW)
        )
        in_maps.append(
            {
                "xiQ": xiQ,
                "x0T": x0p.astype(np.float16),
                "x0Q8": x0p.astype(f8),
                "w1bQ": w1bQ,
                "w1aT": w1aT,
                "w2Q": w2Q,
                "w0Q": w0Q,
                "bmaskT": bmask,
            }
        )
    return in_maps


def _gather(results):
    out = np.empty((B, F), dtype=np.float32)
    for i in range(N_CORES):
        out[i * BL : (i + 1) * BL] = results[i]["res"]
    return out


def kernel(x, W1, W2, W0):
    nc = _get_nc()
    in_maps = _make_in_maps(x, W1, W2, W0)
    res = run_bass_kernel_spmd(nc, in_maps, list(range(N_CORES)))
    return _gather(res.results)


def kernel_profiled(x, W1, W2, W0, **trace_kwargs):
    """Like kernel() but with NTFF profiling; returns (out, exec_time_ns)."""
    nc = _get_nc()
    in_maps = _make_in_maps(x, W1, W2, W0)
    res = run_bass_kernel_spmd(
        nc, in_maps, list(range(N_CORES)), trace=True, **trace_kwargs
    )
    return _gather(res.results), res.exec_time_ns


# revision 23
# speedup vs baseline: 1.1958x; 1.1958x over previous
"""Trainium2 Bass kernel for nn_Attention_39934605918652.

res[b] = W0 @ x0[b] + sum_{n=1..N-1} W2 @ tanh(W1a @ x0[b] + W1b @ x[b,n])

Key algebraic optimization: W2 does not depend on n, so
    sum_n W2 @ tanh(...) = W2 @ (sum_n tanh(...))
which removes the second big matmul (only a [B,H]x[H,F] remains).

Sharding: data-parallel over batch B=128 across 8 cores (16 batches/core),
weights replicated. No collectives.

The dominant [F=512]-contraction matmul runs in fp8 e4m3 DoubleRow mode
(256 contraction rows per instruction; measured 215ns per 512-col matmul
warm = 2x bf16). W1b is host-scaled by 32 so its N(0, 1/1024) entries use
the e4m3 range; the tanh compensates via the ACT scale=1/32 immediate.

Engine budget (measured): the PE+ACT pair carries a conserved ~2.5us per
quad-tile (bias via ACT costs 4 small calls = 1.59us ACT; bias via a K=16
one-hot PE matmul costs 0.63us PE + 1.04us big-call ACT), so the mix knob
KB_NS1 balances them. ACT is not subject to the PE's HAM clock throttle,
so the default mix makes ACT the steady-state pacer. The segmented
free-dim reduce runs on DVE, with a GpSimd halving pre-add (SBUF-only;
GPSIMD cannot touch PSUM, cannot reduce, runs elementwise at 0.42 eff)
offloading KB_NGH of the 32 tiles.

All DRAM tensors are host-packed so every SBUF tile loads with ONE
contiguous dma_start of >=2KB-per-partition rows (small descriptors
measured ~40% DMA throughput loss):
  xiQ   [8*128, 2048] fp8   row (fp*4+q)*128+p = xi[f=fp*256+i*128+p,
                            q*1024+c] pairs (i,c)-major; pad col n=255
  w1bQ  [2*128, 2048] fp8   (= 32*W1b.T, DoubleRow pair layout per fp)
  x0T   [128, 4*16]   fp16  host-packed f-chunks side by side
  x0Q8  [128, 4*16]   fp8   same, for the fp8 W0-term matmuls
  w1aT  [512, 1024]   fp16  (= W1a.T)
  w2Q   [4*128, 1024] fp16  h-tile pairs side by side (= W2.T regrouped)
  w0Q   [128, 2048]   fp8   f-chunks side by side (= W0.T regrouped)
  bmask [16, 4*1024]  fp16  one-hot bias mask per quad: bmask[r, q*1024+
                            b*256+n] = 32 iff r == q*4+b and n != 255
Output res [BL=16, F=512] per core (batch-major); host concatenates.
"""

import os
import numpy as np
from contextlib import ExitStack

import concourse.bass as bass
import concourse.tile as tile
from concourse import bacc, mybir
from concourse.bass_utils import run_bass_kernel_spmd

N_CORES = 8
B, N, F, H = 128, 256, 512, 1024
BL = B // N_CORES          # 16 batches per core
NI = N - 1                 # 255 real columns per batch
NP = 256                   # padded columns per batch
NF = F // 128              # 4 f-chunks
FP = 2                     # 2 f-pair chunks (256 rows each, DoubleRow)
NH = H // 128              # 8 h-tiles
QUADS = BL // 4            # 4 batch-quads; per quad psum tile [128, 4*256]
QW = 4 * NP                # 1024 columns per quad
WSCALE = 32.0              # host-side W1b/bias scale (ACT scale=1/32)

F32 = mybir.dt.float32
BF16 = mybir.dt.bfloat16
F16 = mybir.dt.float16
F8 = mybir.dt.float8e4
DR = mybir.MatmulPerfMode.DoubleRow

# Knobs (sweepable on hw):
#  KB_NS1A: tiles using S1a = 4 per-batch fused-bias ACT calls with the
#          ACT accumulator producing S directly (no reduce; +187ns engine
#          time per call for the accumulator read).
#  KB_NS1: tiles using S1 = 4 per-batch fused-bias ACT calls + reduce.
#          Remaining tiles use S4 = PE one-hot bias matmul + one big
#          1024-col plain tanh + reduce. The last TAIL_S4 tiles are
#          forced S4 (single tanh call drains the pipeline fastest).
#  KB_NGH: reduces prefaced by a GpSimd halving add, then a half-width
#          DVE reduce. Rest: plain DVE reduce_sum.
#  KB_NTTR: DVE tensor_tensor_reduce variant — measured on hw: the
#          instruction wedges the device (NRT unrecoverable). Leave 0.
#  KB_WARM: dummy [128,128] f32 matmuls (426ns each: fp32 = 2 passes)
#          during the DMA lead-in to keep the PE clock governor warm.
#  KB_PPB: main PSUM pool bufs ([128,1024] f32 slots, 2 banks each).
NS1A = int(os.environ.get("KB_NS1A", "0"))
NS1 = int(os.environ.get("KB_NS1", "16"))
NS2D = int(os.environ.get("KB_NS2D", "0"))
NTTR = int(os.environ.get("KB_NTTR", "0"))
NGH = int(os.environ.get("KB_NGH", "22"))
WARM_N = int(os.environ.get("KB_WARM", "9"))
PPB = int(os.environ.get("KB_PPB", "4"))
TAIL_S4 = int(os.environ.get("KB_TAIL", "5"))


def _build_kernel():
    nc = bacc.Bacc(
        "TRN2", target_bir_lowering=False, debug=False, num_devices=N_CORES
    )

    xiQ = nc.dram_tensor("xiQ", [FP * QUADS * 128, 2048], F8, kind="ExternalInput").ap()
    w1bQ = nc.dram_tensor("w1bQ", [FP * 128, 2048], F8, kind="ExternalInput").ap()
    x0T = nc.dram_tensor("x0T", [128, NF * BL], F16, kind="ExternalInput").ap()
    x0Q8 = nc.dram_tensor("x0Q8", [128, NF * BL], F8, kind="ExternalInput").ap()
    w1aT = nc.dram_tensor("w1aT", [F, H], F16, kind="ExternalInput").ap()
    w2Q = nc.dram_tensor("w2Q", [NF * 128, 1024], F16, kind="ExternalInput").ap()
    w0Q = nc.dram_tensor("w0Q", [128, 2048], F8, kind="ExternalInput").ap()
    bmaskT = nc.dram_tensor(
        "bmaskT", [BL, QUADS * 1024], F16, kind="ExternalInput"
    ).ap()
    res = nc.dram_tensor("res", [BL, F], F32, kind="ExternalOutput").ap()

    with tile.TileContext(nc) as tc:
        with ExitStack() as ctx:
            _kernel_body(
                ctx, tc, xiQ, w1bQ, x0T, x0Q8, w1aT, w2Q, w0Q, bmaskT, res
            )

    nc.compile()
    return nc


def _kernel_body(ctx, tc, xiQ, w1bQ, x0T, x0Q8, w1aT, w2Q, w0Q, bmaskT, res):
    nc = tc.nc
    Tanh = mybir.ActivationFunctionType.Tanh

    wpool = ctx.enter_context(tc.tile_pool(name="weights", bufs=1))

    def load_rows(name, dram, r0, shape, dt):
        t = wpool.tile(shape, dt, tag=name, name=name)
        flat = t[:] if len(shape) == 2 else t[:].rearrange("p a b -> p (a b)")
        nc.sync.dma_start(flat, dram[r0 : r0 + shape[0], :])
        return t

    # ---- DMA issue order = first-need order: the very first tile's
    # operands stream before anything else so the PE starts ~9.5us.
    xi_sb = [[None] * QUADS for _ in range(FP)]
    w1b_sb = [None, None]
    w1b_sb[0] = load_rows("w1b_0", w1bQ, 0, [128, 2, 1024], F8)
    xi_sb[0][0] = load_rows("xi_0_0", xiQ, 0, [128, 2, 1024], F8)
    w1b_sb[1] = load_rows("w1b_1", w1bQ, 128, [128, 2, 1024], F8)
    xi_sb[1][0] = load_rows("xi_1_0", xiQ, QUADS * 128, [128, 2, 1024], F8)
    x0_all = load_rows("x0", x0T, 0, [128, NF * BL], F16)
    x0_sb = [x0_all[:, f * BL : (f + 1) * BL] for f in range(NF)]
    x08_all = load_rows("x08", x0Q8, 0, [128, NF * BL], F8)
    x08_sb = [x08_all[:, f * BL : (f + 1) * BL] for f in range(NF)]
    for fp in range(FP):
        xi_sb[fp][1] = load_rows(
            f"xi_{fp}_1", xiQ, (fp * QUADS + 1) * 128, [128, 2, 1024], F8
        )
    w1a_sb = [
        load_rows(f"w1a_{c}", w1aT, c * 128, [128, H], F16) for c in range(NF)
    ]
    bmask_sb = load_rows("bmask", bmaskT, 0, [BL, QUADS * 1024], F16)
    w0_sb = load_rows("w0", w0Q, 0, [128, 2048], F8)
    for q in (2, 3):  # wave-1 quads stream during wave 0
        for fp in range(FP):
            xi_sb[fp][q] = load_rows(
                f"xi_{fp}_{q}", xiQ, (fp * QUADS + q) * 128, [128, 2, 1024], F8
            )
    w2_sb = [
        load_rows(f"w2_{j}", w2Q, j * 128, [128, 1024], F16) for j in range(NF)
    ]

    def w2_slice(h):
        return w2_sb[h // 2][:, (h % 2) * 512 : (h % 2 + 1) * 512]

    h0_sb = [
        wpool.tile([128, BL], F32, tag=f"h0_{h}", name=f"h0_{h}")
        for h in range(NH)
    ]
    h0T_sb = wpool.tile([BL, H], F16, tag="h0T", name="h0T")
    h0s_sb = [
        wpool.tile([128, BL], F32, tag=f"h0s_{h}", name=f"h0s_{h}")
        for h in range(NH)
    ]
    S_sb = [
        wpool.tile([128, BL], F16, tag=f"S_{h}", name=f"S_{h}")
        for h in range(NH)
    ]

    # One PSUM pool; every tile shares the tag so slots recycle.
    # Slot = [128, 4*NP] f32 = 2 banks; PPB slots = the full 8 banks.
    # Warm-up, phase 1, and the epilogue matmul groups borrow slots
    # transiently; the epilogue accumulates in SBUF via DVE.
    ppool = ctx.enter_context(tc.tile_pool(name="ps", bufs=PPB, space="PSUM"))
    itpool = ctx.enter_context(tc.tile_pool(name="it", bufs=6))

    # ---- Phase 0: PE warm-up during the DMA lead-in ----
    if WARM_N:
        wz = wpool.tile([128, 128], F32, tag="warmz", name="warmz")
        nc.vector.memset(wz[:], 0.0)
        pw = ppool.tile([128, 128], F32, tag="ps", name="pwarm")
        for _ in range(WARM_N):
            nc.tensor.matmul(pw[:], wz[:], wz[:], start=True, stop=True)

    # ---- Phase 0b: preload the tanh ACT table during the DMA lead-in
    tiny = wpool.tile([128, 1], F32, tag="tiny", name="tiny")
    nc.vector.memset(tiny[:], 0.0)
    nc.scalar.activation(tiny[:], tiny[:], Tanh)

    # ---- Phase 1 (issued after the LEAD wave-0 tiles; see below):
    def phase1a():
        # h0[h*128+p, b] = sum_f W1a[h, f] x0[b, f]; [128, BL] per h-tile.
        # One psum tile with a ping-pong sub-range: a fresh pool tile per
        # h would rotate the slot ring and chain each h0 onto a lead
        # tile's consume.
        ph = ppool.tile([128, 2, BL], F32, tag="ps", name="ph0")
        for h in range(NH):
            sl = ph[:, h % 2, :]
            for f in range(NF):
                nc.tensor.matmul(
                    sl,
                    w1a_sb[f][:, h * 128 : (h + 1) * 128],
                    x0_sb[f],
                    start=(f == 0),
                    stop=(f == NF - 1),
                )
            nc.vector.tensor_copy(h0_sb[h][:], sl)
            if NS2D:
                # 32*h0 for the S2d DVE bias path (ACT rescales by 1/32)
                nc.vector.tensor_scalar_mul(h0s_sb[h][:], sl, WSCALE)

    def phase1b():
        # h0T[b, h] flipped variant for the S4 bias matmuls' stationary.
        ph = ppool.tile([BL, H], F32, tag="ps", name="ph0T")
        for hb in range(2):
            for f in range(NF):
                nc.tensor.matmul(
                    ph[:, hb * 512 : (hb + 1) * 512],
                    x0_sb[f],
                    w1a_sb[f][:, hb * 512 : (hb + 1) * 512],
                    start=(f == 0),
                    stop=(f == NF - 1),
                )
        with nc.allow_low_precision(reason="h0T feeds fp16 bias matmul"):
            nc.vector.tensor_copy(h0T_sb[:], ph[:])

    # ---- Phase 3: epilogue res = W0 x0 + W2 S, accumulated in SBUF.
    # Each 4-matmul group borrows a psum slot transiently and DVE folds
    # it into rt_acc, so no slot is held across phase 2.
    rt_acc = wpool.tile([BL, F], F32, tag="rt", name="rt_acc")

    def epilogue_w0():
        pw = ppool.tile([BL, F], F32, tag="ps", name="po_w0")
        for f in range(NF):
            nc.tensor.matmul(
                pw[:],
                x08_sb[f],
                w0_sb[:, f * 512 : (f + 1) * 512],
                start=(f == 0),
                stop=(f == NF - 1),
            )
        nc.vector.tensor_copy(rt_acc[:], pw[:])

    def epilogue_s_group(hs, name):
        pg = ppool.tile([BL, F], F32, tag="ps", name=name)
        for i, h in enumerate(hs):
            nc.tensor.matmul(
                pg[:], S_sb[h][:], w2_slice(h),
                start=(i == 0), stop=(i == len(hs) - 1),
            )
        nc.vector.tensor_add(rt_acc[:], rt_acc[:], pg[:])

    # ---- Phase 2: hi matmul (fp8 DoubleRow) + bias + tanh + reduce ----
    def consume(h, q, pb, cls, red):
        it = itpool.tile([128, 4 * NP], BF16, tag="it", name=f"it_{h}_{q}")
        s1ish = cls in ("s1", "s1a")
        nb = NP if cls == "s4" else NI
        with nc.allow_low_precision(
            reason="S accumulated in 16-bit to feed the 16-bit output matmul"
        ):
            if s1ish:
                # per-batch ACT, bias via the ACT bias port, skip pad col;
                # s1a also reads the ACT accumulator = the row sum = S col.
                for bl in range(4):
                    b = q * 4 + bl
                    acc = S_sb[h][:, b : b + 1] if cls == "s1a" else None
                    nc.scalar.activation(
                        it[:, bl * NP : bl * NP + NI],
                        pb[:, bl * NP : bl * NP + NI],
                        Tanh,
                        bias=h0_sb[h][:, b : b + 1],
                        scale=1.0 / WSCALE,
                        accum_out=acc,
                    )
                if cls == "s1a":
                    return
            elif cls == "s2d":
                # S2d: bias on DVE (scalar_tensor_tensor, per-partition
                # scalar = 32*h0 column), then one big strided tanh call.
                for bl in range(4):
                    b = q * 4 + bl
                    # in1 is bypassed; walrus allows only one PSUM input,
                    # so point it at any resident SBUF tile.
                    nc.vector.scalar_tensor_tensor(
                        it[:, bl * NP : bl * NP + NI],
                        pb[:, bl * NP : bl * NP + NI],
                        h0s_sb[h][:, b : b + 1],
                        w1a_sb[0][:, :NI],
                        mybir.AluOpType.add,
                        mybir.AluOpType.bypass,
                    )
                sview = it[:].rearrange("p (b n) -> p b n", b=4)[:, :, :NI]
                nc.scalar.activation(sview, sview, Tanh, scale=1.0 / WSCALE)
            else:
                # S4: bias already in PSUM (one-hot matmul, pad col exact 0
                # since bmask zeroes it and tanh(0)=0): one big tanh call.
                nc.scalar.activation(it[:], pb[:], Tanh, scale=1.0 / WSCALE)
            scol = S_sb[h][:, q * 4 : (q + 1) * 4]
            view = it[:].rearrange("p (b n) -> p b n", b=4)
            if red == "gph":
                # GpSimd halving add (SBUF-only), then half-width DVE reduce.
                hb = nb // 2
                nc.gpsimd.tensor_add(
                    view[:, :, :hb],
                    view[:, :, :hb],
                    view[:, :, nb - hb : nb],
                )
                nc.vector.reduce_sum(
                    scol, view[:, :, : nb - hb], axis=mybir.AxisListType.X
                )
            else:
                nc.vector.reduce_sum(
                    scol, view[:, :, :nb], axis=mybir.AxisListType.X
                )

    def mm_main(pb, h, q, s1ish):
        # 2 DoubleRow matmuls per 512-col block: fpair 0 starts, fpair 1
        # accumulates; S4 groups stay open for the bias matmul.
        for bk in range(2):
            out = pb[:, bk * 512 : (bk + 1) * 512]
            for fp in range(FP):
                nc.tensor.matmul(
                    out,
                    w1b_sb[fp][:, :, h * 128 : (h + 1) * 128],
                    xi_sb[fp][q][:, :, bk * 512 : (bk + 1) * 512],
                    start=(fp == 0),
                    stop=(fp == FP - 1) and s1ish,
                    perf_mode=DR,
                )

    def mm_bias(pb, h, q):
        for bk in range(2):
            nc.tensor.matmul(
                pb[:, bk * 512 : (bk + 1) * 512],
                h0T_sb[:, h * 128 : (h + 1) * 128],
                bmask_sb[:, q * 1024 + bk * 512 : q * 1024 + (bk + 1) * 512],
                start=False,
                stop=True,
            )

    # PE program order: LEAD wave-0 tiles first (gated only on w1b +
    # xi-q0/q1, ~1MB of DMA), then phase 1 (w1a streams meanwhile), then
    # the rest. Wave 1 walks h DESCENDING so S[7..4] complete early and
    # their epilogue group issues mid-stream; only {3..0} trails the
    # final consume.
    sched = []
    for wave in range(QUADS // 2):
        hs = range(NH) if wave == 0 else range(NH - 1, -1, -1)
        for h in hs:
            for q in (2 * wave, 2 * wave + 1):
                sched.append((wave, h, q))

    LEAD = 3  # <= PPB-1: lead tiles hold slots; phase 1 borrows the last

    # Deficit-spread class labels; first LEAD tiles non-S4 (their matmuls
    # precede h0T in the PE stream), last TAIL_S4 tiles forced S4.
    counts = {"s1a": NS1A, "s1": NS1, "s2d": NS2D}
    nons4 = counts["s1a"] + counts["s1"] + counts["s2d"]
    if nons4 < LEAD:
        counts["s1"] += LEAD - nons4
    counts["s4"] = 32 - sum(counts.values())
    labels = []
    used = {k: 0 for k in counts}
    for pos in range(32):
        opts = [k for k in counts if used[k] < counts[k]]
        if pos < LEAD:
            opts = [k for k in opts if k != "s4"] or ["s1a"]
        elif pos >= 32 - TAIL_S4 and used["s4"] < counts["s4"]:
            opts = ["s4"]
        pick = max(opts, key=lambda k: counts[k] * (pos + 1) / 32 - used[k])
        used[pick] = used.get(pick, 0) + 1
        labels.append(pick)
    nred = sum(1 for l in labels if l != "s1a")
    rcounts = {"ttr": min(NTTR, nred), "gph": min(NGH, max(nred - NTTR, 0))}
    rcounts["plain"] = nred - rcounts["ttr"] - rcounts["gph"]
    rlabels = []
    rused = {k: 0 for k in rcounts}
    for pos in range(nred):
        opts = [k for k in rcounts if rused[k] < rcounts[k]]
        if pos >= nred - 4 and rused["plain"] < rcounts["plain"]:
            opts = ["plain"]
        pick = max(opts, key=lambda k: rcounts[k] * (pos + 1) / nred - rused[k])
        rused[pick] += 1
        rlabels.append(pick)
    riter = iter(rlabels)
    plan = [(l, next(riter) if l != "s1a" else None) for l in labels]

    deferred = []
    for pos, (wave, h, q) in enumerate(sched):
        if pos == LEAD:
            phase1a()
            phase1b()
            for dh, dq, dpb, dcls, dred in deferred:
                if dcls == "s4":
                    mm_bias(dpb, dh, dq)
                consume(dh, dq, dpb, dcls, dred)
        cls, red = plan[pos]
        pb = ppool.tile([128, 4 * NP], F32, tag="ps", name=f"pb_{h}_{q}")
        mm_main(pb, h, q, cls != "s4")
        if pos < LEAD:
            deferred.append((h, q, pb, cls, red))
            continue
        if cls == "s4":
            mm_bias(pb, h, q)
        consume(h, q, pb, cls, red)
        if pos == 11:
            epilogue_w0()
        if pos == 25:
            epilogue_s_group([7, 6, 5, 4], "po_sA")
        if pos == 29:
            epilogue_s_group([3, 2], "po_sB1")

    epilogue_s_group([1, 0], "po_sB2")
    nc.sync.dma_start(res[:], rt_acc[:])


_NC_CACHE = {}


def _get_nc():
    key = ("v10", NS1A, NS1, NS2D, NTTR, NGH, WARM_N, PPB, TAIL_S4)
    if key not in _NC_CACHE:
        _NC_CACHE[key] = _build_kernel()
    return _NC_CACHE[key]


def _make_in_maps(x, W1, W2, W0):
    import ml_dtypes

    f8 = ml_dtypes.float8_e4m3
    x = np.ascontiguousarray(np.asarray(x, dtype=np.float32))
    W1 = np.asarray(W1, dtype=np.float32)
    W2 = np.asarray(W2, dtype=np.float32)
    W0 = np.asarray(W0, dtype=np.float32)

    w1aT = np.ascontiguousarray(W1[:, :F].T).astype(np.float16)       # [F, H]
    w1bT = (W1[:, F:].T * WSCALE).astype(f8)                          # [F, H]
    # DoubleRow pair layout: row fp*128+p = [i0 h0..1023, i1 h0..1023]
    w1bQ = np.ascontiguousarray(
        w1bT.reshape(FP, 2, 128, H).transpose(0, 2, 1, 3).reshape(FP * 128, 2 * H)
    )
    w2T = np.ascontiguousarray(W2.T).astype(np.float16)               # [H, F]
    w2Q = np.ascontiguousarray(
        w2T.reshape(NF, 2, 128, F).transpose(0, 2, 1, 3).reshape(NF * 128, 2 * F)
    )
    w0T = np.ascontiguousarray(W0.T).astype(f8)                       # [F, F]
    w0Q = np.ascontiguousarray(
        w0T.reshape(NF, 128, F).transpose(1, 0, 2).reshape(128, NF * F)
    )

    # bmask[r, q*1024 + b*256 + n] = WSCALE iff r == q*4+b and n != 255
    bmask = np.zeros((BL, QUADS, 4, NP), dtype=np.float16)
    for qq in range(QUADS):
        for bb in range(4):
            bmask[qq * 4 + bb, qq, bb, :NI] = WSCALE
    bmask = bmask.reshape(BL, QUADS * 1024)

    in_maps = []
    for i in range(N_CORES):
        xc = x[i * BL : (i + 1) * BL]               # [BL, N, F]
        # packed [128, NF*BL]: row p, block f holds x0T[f*128+p, :]
        x0p = np.ascontiguousarray(
            xc[:, 0, :].T.reshape(NF, 128, BL).transpose(1, 0, 2).reshape(128, NF * BL)
        )
        pad = np.zeros((BL, NP, F), dtype=np.float32)
        pad[:, :NI, :] = xc[:, 1:, :]
        xiT = pad.reshape(BL * NP, F).T.astype(f8)  # [F, BL*NP]
        # row (fp*4+q)*128+p = [i0 c0..1023, i1 c0..1023] of quad q
        xiQ = np.ascontiguousarray(
            xiT.reshape(FP, 2, 128, QUADS, QW)
            .transpose(0, 3, 2, 1, 4)
            .reshape(FP * QUADS * 128, 2 * QW)
        )
        in_maps.append(
            {
                "xiQ": xiQ,
                "x0T": x0p.astype(np.float16),
                "x0Q8": x0p.astype(f8),
                "w1bQ": w1bQ,
                "w1aT": w1aT,
                "w2Q": w2Q,
                "w0Q": w0Q,
                "bmaskT": bmask,
            }
        )
    return in_maps


def _gather(results):
    out = np.empty((B, F), dtype=np.float32)
    for i in range(N_CORES):
        out[i * BL : (i + 1) * BL] = results[i]["res"]
    return out


def kernel(x, W1, W2, W0):
    nc = _get_nc()
    in_maps = _make_in_maps(x, W1, W2, W0)
    res = run_bass_kernel_spmd(nc, in_maps, list(range(N_CORES)))
    return _gather(res.results)


def kernel_profiled(x, W1, W2, W0, **trace_kwargs):
    """Like kernel() but with NTFF profiling; returns (out, exec_time_ns)."""
    nc = _get_nc()
    in_maps = _make_in_maps(x, W1, W2, W0)
    res = run_bass_kernel_spmd(
        nc, in_maps, list(range(N_CORES)), trace=True, **trace_kwargs
    )
    return _gather(res.results), res.exec_time_ns


# revision 25
# speedup vs baseline: 1.2181x; 1.0187x over previous
"""Trainium2 Bass kernel for nn_Attention_39934605918652.

res[b] = W0 @ x0[b] + sum_{n=1..N-1} W2 @ tanh(W1a @ x0[b] + W1b @ x[b,n])

Key algebraic optimization: W2 does not depend on n, so
    sum_n W2 @ tanh(...) = W2 @ (sum_n tanh(...))
which removes the second big matmul (only a [B,H]x[H,F] remains).

Sharding: data-parallel over batch B=128 across 8 cores (16 batches/core),
weights replicated. No collectives.

The dominant [F=512]-contraction matmul runs in fp8 e4m3 DoubleRow mode
(256 contraction rows per instruction; measured 215ns per 512-col matmul
warm = 2x bf16). W1b is host-scaled by 32 so its N(0, 1/1024) entries use
the e4m3 range; the tanh compensates via the ACT scale=1/32 immediate.

Engine budget (measured): the PE+ACT pair carries a conserved ~2.5us per
quad-tile (bias via ACT costs 4 small calls = 1.59us ACT; bias via a K=16
one-hot PE matmul costs 0.63us PE + 1.04us big-call ACT), so the mix knob
KB_NS1 balances them. ACT is not subject to the PE's HAM clock throttle,
so the default mix makes ACT the steady-state pacer. The segmented
free-dim reduce runs on DVE, with a GpSimd halving pre-add (SBUF-only;
GPSIMD cannot touch PSUM, cannot reduce, runs elementwise at 0.42 eff)
offloading KB_NGH of the 32 tiles.

All DRAM tensors are host-packed so every SBUF tile loads with ONE
contiguous dma_start of >=2KB-per-partition rows (small descriptors
measured ~40% DMA throughput loss):
  xiQ   [8*128, 2048] fp8   row (fp*4+q)*128+p = xi[f=fp*256+i*128+p,
                            q*1024+c] pairs (i,c)-major; pad col n=255
  w1bQ  [2*128, 2048] fp8   (= 32*W1b.T, DoubleRow pair layout per fp)
  x0T   [128, 4*16]   fp16  host-packed f-chunks side by side
  x0Q8  [128, 4*16]   fp8   same, for the fp8 W0-term matmuls
  w1aT  [512, 1024]   fp16  (= W1a.T)
  w2Q   [4*128, 1024] fp16  h-tile pairs side by side (= W2.T regrouped)
  w0Q   [128, 2048]   fp8   f-chunks side by side (= W0.T regrouped)
  bmask [16, 4*1024]  fp16  one-hot bias mask per quad: bmask[r, q*1024+
                            b*256+n] = 32 iff r == q*4+b and n != 255
Output res [BL=16, F=512] per core (batch-major); host concatenates.
"""

import os
import numpy as np
from contextlib import ExitStack

import concourse.bass as bass
import concourse.tile as tile
from concourse import bacc, mybir
from concourse.bass_utils import run_bass_kernel_spmd

N_CORES = 8
B, N, F, H = 128, 256, 512, 1024
BL = B // N_CORES          # 16 batches per core
NI = N - 1                 # 255 real columns per batch
NP = 256                   # padded columns per batch
NF = F // 128              # 4 f-chunks
FP = 2                     # 2 f-pair chunks (256 rows each, DoubleRow)
NH = H // 128              # 8 h-tiles
QUADS = BL // 4            # 4 batch-quads; per quad psum tile [128, 4*256]
QW = 4 * NP                # 1024 columns per quad
WSCALE = 32.0              # host-side W1b/bias scale (ACT scale=1/32)

F32 = mybir.dt.float32
BF16 = mybir.dt.bfloat16
F16 = mybir.dt.float16
F8 = mybir.dt.float8e4
DR = mybir.MatmulPerfMode.DoubleRow

# Knobs (sweepable on hw):
#  KB_NS1A: tiles using S1a = 4 per-batch fused-bias ACT calls with the
#          ACT accumulator producing S directly (no reduce; +187ns engine
#          time per call for the accumulator read).
#  KB_NS1: tiles using S1 = 4 per-batch fused-bias ACT calls + reduce.
#          Remaining tiles use S4 = PE one-hot bias matmul + one big
#          1024-col plain tanh + reduce. The last TAIL_S4 tiles are
#          forced S4 (single tanh call drains the pipeline fastest).
#  KB_NGH: reduces prefaced by a GpSimd halving add, then a half-width
#          DVE reduce. Rest: plain DVE reduce_sum.
#  KB_NTTR: DVE tensor_tensor_reduce variant — measured on hw: the
#          instruction wedges the device (NRT unrecoverable). Leave 0.
#  KB_WARM: dummy [128,128] f32 matmuls (426ns each: fp32 = 2 passes)
#          during the DMA lead-in to keep the PE clock governor warm.
#  KB_PPB: main PSUM pool bufs ([128,1024] f32 slots, 2 banks each).
NS1A = int(os.environ.get("KB_NS1A", "0"))
NS1 = int(os.environ.get("KB_NS1", "16"))
NS2D = int(os.environ.get("KB_NS2D", "0"))
NTTR = int(os.environ.get("KB_NTTR", "0"))
NGH = int(os.environ.get("KB_NGH", "22"))
WARM_N = int(os.environ.get("KB_WARM", "9"))
PPB = int(os.environ.get("KB_PPB", "4"))
TAIL_S4 = int(os.environ.get("KB_TAIL", "5"))


def _build_kernel():
    nc = bacc.Bacc(
        "TRN2", target_bir_lowering=False, debug=False, num_devices=N_CORES
    )

    xiQ = nc.dram_tensor("xiQ", [FP * QUADS * 128, 2048], F8, kind="ExternalInput").ap()
    w1bQ = nc.dram_tensor("w1bQ", [FP * 128, 2048], F8, kind="ExternalInput").ap()
    x0T = nc.dram_tensor("x0T", [128, NF * BL], F16, kind="ExternalInput").ap()
    x0Q8 = nc.dram_tensor("x0Q8", [128, NF * BL], F8, kind="ExternalInput").ap()
    w1aT = nc.dram_tensor("w1aT", [F, H], F16, kind="ExternalInput").ap()
    w2Q = nc.dram_tensor("w2Q", [NF * 128, 1024], F16, kind="ExternalInput").ap()
    w0Q = nc.dram_tensor("w0Q", [128, 2048], F8, kind="ExternalInput").ap()
    bmaskT = nc.dram_tensor(
        "bmaskT", [BL, QUADS * 1024], F16, kind="ExternalInput"
    ).ap()
    res = nc.dram_tensor("res", [BL, F], F32, kind="ExternalOutput").ap()

    with tile.TileContext(nc) as tc:
        with ExitStack() as ctx:
            _kernel_body(
                ctx, tc, xiQ, w1bQ, x0T, x0Q8, w1aT, w2Q, w0Q, bmaskT, res
            )

    nc.compile()
    return nc


def _kernel_body(ctx, tc, xiQ, w1bQ, x0T, x0Q8, w1aT, w2Q, w0Q, bmaskT, res):
    nc = tc.nc
    Tanh = mybir.ActivationFunctionType.Tanh

    wpool = ctx.enter_context(tc.tile_pool(name="weights", bufs=1))

    def load_rows(name, dram, r0, shape, dt):
        t = wpool.tile(shape, dt, tag=name, name=name)
        flat = t[:] if len(shape) == 2 else t[:].rearrange("p a b -> p (a b)")
        nc.sync.dma_start(flat, dram[r0 : r0 + shape[0], :])
        return t

    # ---- DMA issue order = first-need order: the very first tile's
    # operands stream before anything else so the PE starts ~9.5us.
    xi_sb = [[None] * QUADS for _ in range(FP)]
    w1b_sb = [None, None]
    w1b_sb[0] = load_rows("w1b_0", w1bQ, 0, [128, 2, 1024], F8)
    xi_sb[0][0] = load_rows("xi_0_0", xiQ, 0, [128, 2, 1024], F8)
    w1b_sb[1] = load_rows("w1b_1", w1bQ, 128, [128, 2, 1024], F8)
    xi_sb[1][0] = load_rows("xi_1_0", xiQ, QUADS * 128, [128, 2, 1024], F8)
    x0_all = load_rows("x0", x0T, 0, [128, NF * BL], F16)
    x0_sb = [x0_all[:, f * BL : (f + 1) * BL] for f in range(NF)]
    x08_all = load_rows("x08", x0Q8, 0, [128, NF * BL], F8)
    x08_sb = [x08_all[:, f * BL : (f + 1) * BL] for f in range(NF)]
    for fp in range(FP):
        xi_sb[fp][1] = load_rows(
            f"xi_{fp}_1", xiQ, (fp * QUADS + 1) * 128, [128, 2, 1024], F8
        )
    w1a_sb = [
        load_rows(f"w1a_{c}", w1aT, c * 128, [128, H], F16) for c in range(NF)
    ]
    bmask_sb = load_rows("bmask", bmaskT, 0, [BL, QUADS * 1024], F16)
    w0_sb = load_rows("w0", w0Q, 0, [128, 2048], F8)
    for q in (2, 3):  # wave-1 quads stream during wave 0
        for fp in range(FP):
            xi_sb[fp][q] = load_rows(
                f"xi_{fp}_{q}", xiQ, (fp * QUADS + q) * 128, [128, 2, 1024], F8
            )
    w2_sb = [
        load_rows(f"w2_{j}", w2Q, j * 128, [128, 1024], F16) for j in range(NF)
    ]

    def w2_slice(h):
        return w2_sb[h // 2][:, (h % 2) * 512 : (h % 2 + 1) * 512]

    h0_sb = [
        wpool.tile([128, BL], F32, tag=f"h0_{h}", name=f"h0_{h}")
        for h in range(NH)
    ]
    h0T_sb = wpool.tile([BL, H], F16, tag="h0T", name="h0T")
    h0s_sb = [
        wpool.tile([128, BL], F32, tag=f"h0s_{h}", name=f"h0s_{h}")
        for h in range(NH)
    ]
    S_sb = [
        wpool.tile([128, BL], F16, tag=f"S_{h}", name=f"S_{h}")
        for h in range(NH)
    ]

    # One PSUM pool; every tile shares the tag so slots recycle.
    # Slot = [128, 4*NP] f32 = 2 banks; PPB slots = the full 8 banks.
    # Warm-up, phase 1, and the epilogue matmul groups borrow slots
    # transiently; the epilogue accumulates in SBUF via DVE.
    ppool = ctx.enter_context(tc.tile_pool(name="ps", bufs=PPB, space="PSUM"))
    itpool = ctx.enter_context(tc.tile_pool(name="it", bufs=6))

    # ---- Phase 0: PE warm-up during the DMA lead-in ----
    if WARM_N:
        wz = wpool.tile([128, 128], F32, tag="warmz", name="warmz")
        nc.vector.memset(wz[:], 0.0)
        pw = ppool.tile([128, 128], F32, tag="ps", name="pwarm")
        for _ in range(WARM_N):
            nc.tensor.matmul(pw[:], wz[:], wz[:], start=True, stop=True)

    # ---- Phase 0b: preload the tanh ACT table during the DMA lead-in
    tiny = wpool.tile([128, 1], F32, tag="tiny", name="tiny")
    nc.vector.memset(tiny[:], 0.0)
    nc.scalar.activation(tiny[:], tiny[:], Tanh)

    # ---- Phase 1 (issued after the LEAD wave-0 tiles; see below):
    def phase1a():
        # h0[h*128+p, b] = sum_f W1a[h, f] x0[b, f]; [128, BL] per h-tile.
        # ONE psum tile holding all 8 h sub-ranges, f-OUTER so the PE
        # starts as soon as the first w1a chunk lands (f-inner would gate
        # every h0 on the LAST w1a DMA); a fresh pool tile per h would
        # also rotate the slot ring and chain h0 onto a lead consume.
        ph = ppool.tile([128, NH, BL], F32, tag="ps", name="ph0")
        # 8 interleaved accumulation groups would fight over the bank's
        # zero region; zero once and accumulate with start=False instead.
        nc.vector.memset(ph[:], 0.0)
        for f in range(NF):
            for h in range(NH):
                nc.tensor.matmul(
                    ph[:, h, :],
                    w1a_sb[f][:, h * 128 : (h + 1) * 128],
                    x0_sb[f],
                    start=False,
                    stop=(f == NF - 1),
                    skip_group_check=True,
                )
        for h in range(NH):
            nc.vector.tensor_copy(h0_sb[h][:], ph[:, h, :])
            if NS2D:
                # 32*h0 for the S2d DVE bias path (ACT rescales by 1/32)
                nc.vector.tensor_scalar_mul(h0s_sb[h][:], ph[:, h, :], WSCALE)

    def phase1b():
        # h0T[b, h] flipped variant for the S4 bias matmuls' stationary.
        ph = ppool.tile([BL, H], F32, tag="ps", name="ph0T")
        for f in range(NF):
            for hb in range(2):
                nc.tensor.matmul(
                    ph[:, hb * 512 : (hb + 1) * 512],
                    x0_sb[f],
                    w1a_sb[f][:, hb * 512 : (hb + 1) * 512],
                    start=(f == 0),
                    stop=(f == NF - 1),
                )
        with nc.allow_low_precision(reason="h0T feeds fp16 bias matmul"):
            nc.vector.tensor_copy(h0T_sb[:], ph[:])

    # ---- Phase 3: epilogue res = W0 x0 + W2 S, accumulated in SBUF.
    # Each 4-matmul group borrows a psum slot transiently and DVE folds
    # it into rt_acc, so no slot is held across phase 2.
    rt_acc = wpool.tile([BL, F], F32, tag="rt", name="rt_acc")

    def epilogue_w0():
        pw = ppool.tile([BL, F], F32, tag="ps", name="po_w0")
        for f in range(NF):
            nc.tensor.matmul(
                pw[:],
                x08_sb[f],
                w0_sb[:, f * 512 : (f + 1) * 512],
                start=(f == 0),
                stop=(f == NF - 1),
            )
        nc.vector.tensor_copy(rt_acc[:], pw[:])

    def epilogue_s_group(hs, name):
        pg = ppool.tile([BL, F], F32, tag="ps", name=name)
        for i, h in enumerate(hs):
            nc.tensor.matmul(
                pg[:], S_sb[h][:], w2_slice(h),
                start=(i == 0), stop=(i == len(hs) - 1),
            )
        nc.vector.tensor_add(rt_acc[:], rt_acc[:], pg[:])

    # ---- Phase 2: hi matmul (fp8 DoubleRow) + bias + tanh + reduce ----
    def consume(h, q, pb, cls, red):
        it = itpool.tile([128, 4 * NP], BF16, tag="it", name=f"it_{h}_{q}")
        s1ish = cls in ("s1", "s1a")
        nb = NP if cls == "s4" else NI
        with nc.allow_low_precision(
            reason="S accumulated in 16-bit to feed the 16-bit output matmul"
        ):
            if s1ish:
                # per-batch ACT, bias via the ACT bias port, skip pad col;
                # s1a also reads the ACT accumulator = the row sum = S col.
                for bl in range(4):
                    b = q * 4 + bl
                    acc = S_sb[h][:, b : b + 1] if cls == "s1a" else None
                    nc.scalar.activation(
                        it[:, bl * NP : bl * NP + NI],
                        pb[:, bl * NP : bl * NP + NI],
                        Tanh,
                        bias=h0_sb[h][:, b : b + 1],
                        scale=1.0 / WSCALE,
                        accum_out=acc,
                    )
                if cls == "s1a":
                    return
            elif cls == "s2d":
                # S2d: bias on DVE (scalar_tensor_tensor, per-partition
                # scalar = 32*h0 column), then one big strided tanh call.
                for bl in range(4):
                    b = q * 4 + bl
                    # in1 is bypassed; walrus allows only one PSUM input,
                    # so point it at any resident SBUF tile.
                    nc.vector.scalar_tensor_tensor(
                        it[:, bl * NP : bl * NP + NI],
                        pb[:, bl * NP : bl * NP + NI],
                        h0s_sb[h][:, b : b + 1],
                        w1a_sb[0][:, :NI],
                        mybir.AluOpType.add,
                        mybir.AluOpType.bypass,
                    )
                sview = it[:].rearrange("p (b n) -> p b n", b=4)[:, :, :NI]
                nc.scalar.activation(sview, sview, Tanh, scale=1.0 / WSCALE)
            else:
                # S4: bias already in PSUM (one-hot matmul, pad col exact 0
                # since bmask zeroes it and tanh(0)=0): one big tanh call.
                nc.scalar.activation(it[:], pb[:], Tanh, scale=1.0 / WSCALE)
            scol = S_sb[h][:, q * 4 : (q + 1) * 4]
            view = it[:].rearrange("p (b n) -> p b n", b=4)
            if red == "gph":
                # GpSimd halving add (SBUF-only), then half-width DVE reduce.
                hb = nb // 2
                nc.gpsimd.tensor_add(
                    view[:, :, :hb],
                    view[:, :, :hb],
                    view[:, :, nb - hb : nb],
                )
                nc.vector.reduce_sum(
                    scol, view[:, :, : nb - hb], axis=mybir.AxisListType.X
                )
            else:
                nc.vector.reduce_sum(
                    scol, view[:, :, :nb], axis=mybir.AxisListType.X
                )

    def mm_main(pb, h, q, s1ish):
        # 2 DoubleRow matmuls per 512-col block: fpair 0 starts, fpair 1
        # accumulates; S4 groups stay open for the bias matmul.
        for bk in range(2):
            out = pb[:, bk * 512 : (bk + 1) * 512]
            for fp in range(FP):
                nc.tensor.matmul(
                    out,
                    w1b_sb[fp][:, :, h * 128 : (h + 1) * 128],
                    xi_sb[fp][q][:, :, bk * 512 : (bk + 1) * 512],
                    start=(fp == 0),
                    stop=(fp == FP - 1) and s1ish,
                    perf_mode=DR,
                )

    def mm_bias(pb, h, q):
        for bk in range(2):
            nc.tensor.matmul(
                pb[:, bk * 512 : (bk + 1) * 512],
                h0T_sb[:, h * 128 : (h + 1) * 128],
                bmask_sb[:, q * 1024 + bk * 512 : q * 1024 + (bk + 1) * 512],
                start=False,
                stop=True,
            )

    # PE program order: LEAD wave-0 tiles first (gated only on w1b +
    # xi-q0/q1, ~1MB of DMA), then phase 1 (w1a streams meanwhile), then
    # the rest. Wave 1 walks h DESCENDING so S[7..4] complete early and
    # their epilogue group issues mid-stream; only {3..0} trails the
    # final consume.
    sched = []
    for wave in range(QUADS // 2):
        hs = range(NH) if wave == 0 else range(NH - 1, -1, -1)
        for h in hs:
            for q in (2 * wave, 2 * wave + 1):
                sched.append((wave, h, q))

    LEAD = 3  # <= PPB-1: lead tiles hold slots; phase 1 borrows the last

    # Deficit-spread class labels; first LEAD tiles non-S4 (their matmuls
    # precede h0T in the PE stream), last TAIL_S4 tiles forced S4.
    counts = {"s1a": NS1A, "s1": NS1, "s2d": NS2D}
    nons4 = counts["s1a"] + counts["s1"] + counts["s2d"]
    if nons4 < LEAD:
        counts["s1"] += LEAD - nons4
    counts["s4"] = 32 - sum(counts.values())
    labels = []
    used = {k: 0 for k in counts}
    for pos in range(32):
        opts = [k for k in counts if used[k] < counts[k]]
        if pos < LEAD:
            opts = [k for k in opts if k != "s4"] or ["s1a"]
        elif pos >= 32 - TAIL_S4 and used["s4"] < counts["s4"]:
            opts = ["s4"]
        pick = max(opts, key=lambda k: counts[k] * (pos + 1) / 32 - used[k])
        used[pick] = used.get(pick, 0) + 1
        labels.append(pick)
    nred = sum(1 for l in labels if l != "s1a")
    rcounts = {"ttr": min(NTTR, nred), "gph": min(NGH, max(nred - NTTR, 0))}
    rcounts["plain"] = nred - rcounts["ttr"] - rcounts["gph"]
    rlabels = []
    rused = {k: 0 for k in rcounts}
    for pos in range(nred):
        opts = [k for k in rcounts if rused[k] < rcounts[k]]
        if pos >= nred - 4 and rused["plain"] < rcounts["plain"]:
            opts = ["plain"]
        pick = max(opts, key=lambda k: rcounts[k] * (pos + 1) / nred - rused[k])
        rused[pick] += 1
        rlabels.append(pick)
    riter = iter(rlabels)
    plan = [(l, next(riter) if l != "s1a" else None) for l in labels]

    deferred = []
    for pos, (wave, h, q) in enumerate(sched):
        if pos == LEAD:
            phase1a()
            phase1b()
            for dh, dq, dpb, dcls, dred in deferred:
                if dcls == "s4":
                    mm_bias(dpb, dh, dq)
                consume(dh, dq, dpb, dcls, dred)
        cls, red = plan[pos]
        pb = ppool.tile([128, 4 * NP], F32, tag="ps", name=f"pb_{h}_{q}")
        mm_main(pb, h, q, cls != "s4")
        if pos < LEAD:
            deferred.append((h, q, pb, cls, red))
            continue
        if cls == "s4":
            mm_bias(pb, h, q)
        consume(h, q, pb, cls, red)
        if pos == 11:
            epilogue_w0()
        if pos == 25:
            epilogue_s_group([7, 6, 5, 4], "po_sA")
        if pos == 29:
            epilogue_s_group([3, 2], "po_sB1")

    epilogue_s_group([1, 0], "po_sB2")
    nc.sync.dma_start(res[:], rt_acc[:])


_NC_CACHE = {}


def _get_nc():
    key = ("v12", NS1A, NS1, NS2D, NTTR, NGH, WARM_N, PPB, TAIL_S4)
    if key not in _NC_CACHE:
        _NC_CACHE[key] = _build_kernel()
    return _NC_CACHE[key]


def _make_in_maps(x, W1, W2, W0):
    import ml_dtypes

    f8 = ml_dtypes.float8_e4m3
    x = np.ascontiguousarray(np.asarray(x, dtype=np.float32))
    W1 = np.asarray(W1, dtype=np.float32)
    W2 = np.asarray(W2, dtype=np.float32)
    W0 = np.asarray(W0, dtype=np.float32)

    w1aT = np.ascontiguousarray(W1[:, :F].T).astype(np.float16)       # [F, H]
    w1bT = (W1[:, F:].T * WSCALE).astype(f8)                          # [F, H]
    # DoubleRow pair layout: row fp*128+p = [i0 h0..1023, i1 h0..1023]
    w1bQ = np.ascontiguousarray(
        w1bT.reshape(FP, 2, 128, H).transpose(0, 2, 1, 3).reshape(FP * 128, 2 * H)
    )
    w2T = np.ascontiguousarray(W2.T).astype(np.float16)               # [H, F]
    w2Q = np.ascontiguousarray(
        w2T.reshape(NF, 2, 128, F).transpose(0, 2, 1, 3).reshape(NF * 128, 2 * F)
    )
    w0T = np.ascontiguousarray(W0.T).astype(f8)                       # [F, F]
    w0Q = np.ascontiguousarray(
        w0T.reshape(NF, 128, F).transpose(1, 0, 2).reshape(128, NF * F)
    )

    # bmask[r, q*1024 + b*256 + n] = WSCALE iff r == q*4+b and n != 255
    bmask = np.zeros((BL, QUADS, 4, NP), dtype=np.float16)
    for qq in range(QUADS):
        for bb in range(4):
            bmask[qq * 4 + bb, qq, bb, :NI] = WSCALE
    bmask = bmask.reshape(BL, QUADS * 1024)

    in_maps = []
    for i in range(N_CORES):
        xc = x[i * BL : (i + 1) * BL]               # [BL, N, F]
        # packed [128, NF*BL]: row p, block f holds x0T[f*128+p, :]
        x0p = np.ascontiguousarray(
            xc[:, 0, :].T.reshape(NF, 128, BL).transpose(1, 0, 2).reshape(128, NF * BL)
        )
        pad = np.zeros((BL, NP, F), dtype=np.float32)
        pad[:, :NI, :] = xc[:, 1:, :]
        xiT = pad.reshape(BL * NP, F).T.astype(f8)  # [F, BL*NP]
        # row (fp*4+q)*128+p = [i0 c0..1023, i1 c0..1023] of quad q
        xiQ = np.ascontiguousarray(
            xiT.reshape(FP, 2, 128, QUADS, QW)
            .transpose(0, 3, 2, 1, 4)
            .reshape(FP * QUADS * 128, 2 * QW)
        )
        in_maps.append(
            {
                "xiQ": xiQ,
                "x0T": x0p.astype(np.float16),
                "x0Q8": x0p.astype(f8),
                "w1bQ": w1bQ,
                "w1aT": w1aT,
                "w2Q": w2Q,
                "w0Q": w0Q,
                "bmaskT": bmask,
            }
        )
    return in_maps


def _gather(results):
    out = np.empty((B, F), dtype=np.float32)
    for i in range(N_CORES):
        out[i * BL : (i + 1) * BL] = results[i]["res"]
    return out


def kernel(x, W1, W2, W0):
    nc = _get_nc()
    in_maps = _make_in_maps(x, W1, W2, W0)
    res = run_bass_kernel_spmd(nc, in_maps, list(range(N_CORES)))
    return _gather(res.results)


def kernel_profiled(x, W1, W2, W0, **trace_kwargs):
    """Like kernel() but with NTFF profiling; returns (out, exec_time_ns)."""
    nc = _get_nc()
    in_maps = _make_in_maps(x, W1, W2, W0)
    res = run_bass_kernel_spmd(
        nc, in_maps, list(range(N_CORES)), trace=True, **trace_kwargs
    )
    return _gather(res.results), res.exec_time_ns
